# revision 14
# baseline (speedup 1.0000x reference)
"""Trainium2 Bass kernel for edge-MLP GNN message passing.

Computation (per edge e with endpoints row[e], col[e]):
    h1 = relu([x[row] | x[col]] @ W1 + b1)        # [E, 256]
    h2 = leaky_relu(h1 @ W2 + b2, 0.01)           # [E, 128]
    out = h2 @ W3 + b3                            # [E]

Strategy: data-parallel over edges across 8 NeuronCores. On each core the
row/col feature gathers use the SWDGE dma_gather custom instruction in
transpose mode (bf16), which lands gathered rows directly in [feat, edge]
layout for the TensorEngine matmuls. dma_gather indices are int16, so edges
are bucketed on the host into 4 groups by (row >= 32768, col >= 32768); the
hi-half buckets gather from a base AP offset by 32768 rows.
"""

import numpy as np
import ml_dtypes

import bass_rust
import concourse.bass as bass
import concourse.mybir as mybir
import concourse.tile as tile
from concourse.bass_utils import run_bass_kernel_spmd
from concourse.library_config import mlp as mlp_library
from concourse.tile_rust import add_dep_helper
from concourse.vector_clock import ScopedClock

BF16 = mybir.dt.bfloat16
F32 = mybir.dt.float32
I16 = mybir.dt.int16

N_NODES = 50000
N_FEAT = 128
N_EDGES = 600000
HIDDEN = 256
H2 = 128
NCORES = 8
T_SPLIT = 32768  # int16 index limit +1
GCHUNK = 2048    # edges per dma_gather instruction
MCHUNK = 512     # edges per matmul group (PSUM free-dim limit for f32)
LEAKY = 0.01

PROFILE = False
LAST_EXEC_NS = None
LAST_RESULTS = None

_PATCHED = False


def _patch_tile_drain():
    """Upstream TileContext attaches every global-clock wait to the single
    final InstDrain, but non-EventSemaphore instructions encode at most one
    wait and walrus rejects the overfull drain. Spread the waits over
    dedicated sync wait instructions instead."""
    global _PATCHED
    if _PATCHED:
        return
    _PATCHED = True

    def _wait_cap(inst):
        # walrus encodes 2 sync waits on EventSemaphore, 1 elsewhere
        return 2 if "EventSemaphore" in type(inst).__name__ else 1

    def _split_overfull_waits(self, nc):
        sem_by_name = {}
        for k, h in self.sems.allocated().items():
            sem_by_name[getattr(h, "name", k)] = h
        cur = nc.cur_bb.bb
        for f in nc.m.functions:
            for bb in f.blocks:
                insts = bb.instructions
                i = 0
                while i < len(insts):
                    inst = insts[i]
                    si = inst.sync_info
                    waits = list(si.on_wait) if si is not None else []
                    cap = _wait_cap(inst)
                    if len(waits) <= cap:
                        i += 1
                        continue
                    keep, extra = waits[:cap], waits[cap:]
                    inst.sync_info = bass_rust.SyncInfo(
                        on_wait=keep, on_update=list(si.on_update)
                    )
                    carriers = []
                    for w in extra:
                        assert w.wait_reg is None, "register waits unsupported"
                        nc.engines[inst.engine].wait_ge(
                            sem_by_name[w.ant_name], w.wait_value
                        )
                        carriers.append(cur.instructions.pop())
                    for c in reversed(carriers):
                        insts.insert(i, c)
                    i += 1 + len(carriers)

    def _drain_and_barrier(self, tick_clock, wait_clock):
        nc = self.nc
        drain_inst = nc.sync.drain()
        wait_clock.add_sem_waits(
            drain_inst.ins, ScopedClock({None: tick_clock.global_clock})
        )
        nc.all_engine_barrier()
        _split_overfull_waits(self, nc)
        popped = nc._tile_sem_poison_stack.pop()
        assert popped is self._sem_poison
        nc.clear_and_free_semaphores(list(self.sems.allocated().values()))
        nc.all_engine_barrier()

    tile.TileContext._drain_and_barrier = _drain_and_barrier


def _gather_schedule(cap):
    """Split a bucket capacity (multiple of MCHUNK) into gather sizes."""
    out = [GCHUNK] * (cap // GCHUNK)
    rem = cap % GCHUNK
    if rem:
        out.append(rem)
    return out


_BUILD_CACHE = {}


def _build(caps):
    """Build the SPMD Bass module for per-core bucket capacities `caps`
    (tuple of 4 ints, each a multiple of MCHUNK)."""
    if caps in _BUILD_CACHE:
        return _BUILD_CACHE[caps]
    _patch_tile_drain()

    L = sum(caps)          # padded edges per core
    L16 = L // 16

    nc = bass.Bass("TRN2", target_bir_lowering=False, debug=False,
                   num_devices=NCORES, num_swdge_queues=4)

    xb = nc.dram_tensor("xb", [N_NODES, N_FEAT], BF16, kind="ExternalInput")
    w1a = nc.dram_tensor("w1a", [N_FEAT, HIDDEN], BF16, kind="ExternalInput")
    w1b = nc.dram_tensor("w1b", [N_FEAT, HIDDEN], BF16, kind="ExternalInput")
    w2 = nc.dram_tensor("w2", [HIDDEN, H2], BF16, kind="ExternalInput")
    w3 = nc.dram_tensor("w3", [H2, 1], BF16, kind="ExternalInput")
    b1 = nc.dram_tensor("b1", [N_FEAT, 2], F32, kind="ExternalInput")
    b2 = nc.dram_tensor("b2", [H2, 1], F32, kind="ExternalInput")
    b3 = nc.dram_tensor("b3", [1, 1], F32, kind="ExternalInput")
    idxr = nc.dram_tensor("idxr", [128, L16], I16, kind="ExternalInput")
    idxc = nc.dram_tensor("idxc", [128, L16], I16, kind="ExternalInput")
    y = nc.dram_tensor("y", [1, L], F32, kind="ExternalOutput")

    with tile.TileContext(nc) as tc:
        with (
            tc.tile_pool(name="const", bufs=1) as cpool,
            tc.tile_pool(name="gather", bufs=5) as gpool,
            tc.tile_pool(name="h", bufs=4) as hpool,
            tc.tile_pool(name="out", bufs=4) as opool,
            tc.tile_pool(name="psum", bufs=2, space="PSUM") as ppool,
            tc.tile_pool(name="psumtr", bufs=2, space="PSUM") as ppool_tr,
        ):
            # ---- preload constants ----
            w1a_sb = cpool.tile([N_FEAT, HIDDEN], BF16, tag="w1a")
            nc.sync.dma_start(out=w1a_sb[:], in_=w1a[:])
            w1b_sb = cpool.tile([N_FEAT, HIDDEN], BF16, tag="w1b")
            nc.sync.dma_start(out=w1b_sb[:], in_=w1b[:])
            w2_sb = cpool.tile([HIDDEN // 2, 2 * H2], BF16, tag="w2")
            # W2 is [256, 128] with contraction j on partitions; load as two
            # [128, 128] tiles side by side.
            nc.sync.dma_start(out=w2_sb[:, :H2], in_=w2[:128, :])
            nc.sync.dma_start(out=w2_sb[:, H2:], in_=w2[128:, :])
            w3_sb = cpool.tile([H2, 1], BF16, tag="w3")
            nc.sync.dma_start(out=w3_sb[:], in_=w3[:])
            b1_sb = cpool.tile([N_FEAT, 2], F32, tag="b1")
            nc.sync.dma_start(out=b1_sb[:], in_=b1[:])
            b2_sb = cpool.tile([H2, 1], F32, tag="b2")
            nc.sync.dma_start(out=b2_sb[:], in_=b2[:])
            b3_sb = cpool.tile([1, 1], F32, tag="b3")
            nc.sync.dma_start(out=b3_sb[:], in_=b3[:])
            ident = cpool.tile([128, 128], BF16, tag="ident")
            from concourse.masks import make_identity
            make_identity(nc, ident[:])
            idxr_sb = cpool.tile([128, L16], I16, tag="idxr")
            nc.sync.dma_start(out=idxr_sb[:], in_=idxr[:])
            idxc_sb = cpool.tile([128, L16], I16, tag="idxc")
            nc.sync.dma_start(out=idxc_sb[:], in_=idxc[:])

            # dma_gather runs as Q7 ucode from the mlp library; the reload
            # must execute on Pool before any gather (Tile won't order it
            # by data deps, so wire explicit edges)
            lib_inst = nc.gpsimd.load_library(mlp_library).ins

            # one register per distinct gather size (dma_gather's
            # num_idxs_reg); allocating per-call exhausts Pool registers
            reg_cache = {}
            qctr = [0]

            def gsize_reg(v):
                if v not in reg_cache:
                    reg_cache[v] = nc.gpsimd.to_reg(v)
                return reg_cache[v]

            off = 0
            for b in range(4):
                row_base = xb[:] if b < 2 else xb[T_SPLIT:, :]
                col_base = xb[:] if b % 2 == 0 else xb[T_SPLIT:, :]
                for gsize in _gather_schedule(caps[b]):
                    c0 = off // 16
                    c1 = (off + gsize) // 16
                    rowG = gpool.tile([128, GCHUNK // 128, N_FEAT], BF16,
                                      tag="rowG")
                    g1 = nc.gpsimd.dma_gather(
                        rowG[:, :gsize // 128, :],
                        row_base,
                        idxr_sb[:, c0:c1],
                        num_idxs=gsize,
                        num_idxs_reg=gsize_reg(gsize),
                        elem_size=N_FEAT,
                        transpose=False,
                        single_packet=False,
                        queue_num=qctr[0] % 4,
                    )
                    qctr[0] += 1
                    colG = gpool.tile([128, GCHUNK // 128, N_FEAT], BF16,
                                      tag="colG")
                    g2 = nc.gpsimd.dma_gather(
                        colG[:, :gsize // 128, :],
                        col_base,
                        idxc_sb[:, c0:c1],
                        num_idxs=gsize,
                        num_idxs_reg=gsize_reg(gsize),
                        elem_size=N_FEAT,
                        transpose=False,
                        single_packet=False,
                        queue_num=qctr[0] % 4,
                    )
                    qctr[0] += 1
                    add_dep_helper(g1.ins, lib_inst, sync=False,
                                   reason="gather needs mlp lib")
                    add_dep_helper(g2.ins, lib_inst, sync=False,
                                   reason="gather needs mlp lib")
                    for s in range(gsize // MCHUNK):
                        rowT = hpool.tile([128, MCHUNK], BF16, tag="rowT")
                        colT = hpool.tile([128, MCHUNK], BF16, tag="colT")
                        for i in range(MCHUNK // 128):
                            blk = s * (MCHUNK // 128) + i
                            ptr = ppool_tr.tile([128, 128], BF16, tag="ptr")
                            nc.tensor.transpose(ptr[:], rowG[:, blk, :],
                                                ident[:])
                            dst = rowT[:, i * 128:(i + 1) * 128]
                            if i % 2 == 0:
                                nc.scalar.copy(dst, ptr[:])
                            else:
                                nc.vector.tensor_copy(dst, ptr[:])
                            ptc = ppool_tr.tile([128, 128], BF16, tag="ptr")
                            nc.tensor.transpose(ptc[:], colG[:, blk, :],
                                                ident[:])
                            dst = colT[:, i * 128:(i + 1) * 128]
                            if i % 2 == 0:
                                nc.vector.tensor_copy(dst, ptc[:])
                            else:
                                nc.scalar.copy(dst, ptc[:])
                        rT = rowT[:]
                        cT = colT[:]
                        # ---- layer 1: h1[j, e] ----
                        ps_h1_0 = ppool.tile([128, MCHUNK], F32, tag="ph1a")
                        nc.tensor.matmul(ps_h1_0[:], w1a_sb[:, :128], rT,
                                         start=True, stop=False)
                        nc.tensor.matmul(ps_h1_0[:], w1b_sb[:, :128], cT,
                                         start=False, stop=True)
                        ps_h1_1 = ppool.tile([128, MCHUNK], F32, tag="ph1b")
                        nc.tensor.matmul(ps_h1_1[:], w1a_sb[:, 128:], rT,
                                         start=True, stop=False)
                        nc.tensor.matmul(ps_h1_1[:], w1b_sb[:, 128:], cT,
                                         start=False, stop=True)
                        h1_0 = hpool.tile([128, MCHUNK], BF16, tag="h1a")
                        nc.scalar.activation(
                            h1_0[:], ps_h1_0[:],
                            mybir.ActivationFunctionType.Relu,
                            bias=b1_sb[:, 0:1])
                        h1_1 = hpool.tile([128, MCHUNK], BF16, tag="h1b")
                        nc.vector.tensor_scalar(
                            h1_1[:], ps_h1_1[:], b1_sb[:, 1:2], 0.0,
                            op0=mybir.AluOpType.add,
                            op1=mybir.AluOpType.max)
                        # ---- layer 2: h2[m, e] ----
                        ps_h2 = ppool.tile([128, MCHUNK], F32, tag="ph2")
                        nc.tensor.matmul(ps_h2[:], w2_sb[:, :H2], h1_0[:],
                                         start=True, stop=False)
                        nc.tensor.matmul(ps_h2[:], w2_sb[:, H2:], h1_1[:],
                                         start=False, stop=True)
                        z = hpool.tile([128, MCHUNK], BF16, tag="z")
                        nc.vector.tensor_scalar(
                            z[:], ps_h2[:], b2_sb[:, 0:1], None,
                            op0=mybir.AluOpType.add)
                        h2 = hpool.tile([128, MCHUNK], BF16, tag="h2")
                        nc.vector.scalar_tensor_tensor(
                            h2[:], z[:], LEAKY, z[:],
                            op0=mybir.AluOpType.mult,
                            op1=mybir.AluOpType.max)
                        # ---- layer 3: out[e] ----
                        ps_out = ppool.tile([1, MCHUNK], F32, tag="ph1a")
                        nc.tensor.matmul(ps_out[:], w3_sb[:], h2[:],
                                         start=True, stop=True)
                        o_sb = opool.tile([1, MCHUNK], F32, tag="osb")
                        nc.scalar.activation(
                            o_sb[:], ps_out[:],
                            mybir.ActivationFunctionType.Identity,
                            bias=b3_sb[:, 0:1])
                        nc.sync.dma_start(
                            out=y[:, off + s * MCHUNK: off + (s + 1) * MCHUNK],
                            in_=o_sb[:])
                    off += gsize

    # populate .instr bytes for extended-inst InstISA subclasses (the
    # library-reload op); raw Bass skips this Bacc pass
    from concourse.library_overlay import lower_extended_insts
    lower_extended_insts(nc)

    _BUILD_CACHE[caps] = nc
    return nc


def _wrap16(arr, L):
    """[L] -> [128, L//16] gather-index layout: element i at [i%16, i//16],
    replicated across the 8 GPSIMD core partition groups."""
    w = arr.reshape(L // 16, 16).T  # [16, L//16]
    return np.ascontiguousarray(np.tile(w, (8, 1)))


def kernel(x, edge_index, W1, b1, W2, b2, W3, b3):
    global LAST_EXEC_NS, LAST_RESULTS
    x = np.asarray(x)
    edge_index = np.asarray(edge_index)
    row = edge_index[0].astype(np.int64)
    col = edge_index[1].astype(np.int64)

    # ---- bucket edges by (row_hi, col_hi) ----
    key = (row >= T_SPLIT) * 2 + (col >= T_SPLIT)
    order = np.argsort(key, kind="stable")
    sizes = np.bincount(key, minlength=4)

    per_core_bucket = []
    caps = []
    start = 0
    for b in range(4):
        ids = order[start:start + sizes[b]]
        start += sizes[b]
        share = -(-sizes[b] // NCORES)  # ceil
        cap = -(-share // MCHUNK) * MCHUNK
        caps.append(int(cap))
        padded = np.full(NCORES * cap, -1, dtype=np.int64)
        # distribute contiguous slices of size `share`, then pad each to cap
        for c in range(NCORES):
            sl = ids[c * share:(c + 1) * share]
            padded[c * cap:c * cap + len(sl)] = sl
        per_core_bucket.append(padded.reshape(NCORES, cap))
    caps = tuple(caps)
    L = sum(caps)

    positions = np.concatenate(per_core_bucket, axis=1)  # [NCORES, L]
    assert positions.shape == (NCORES, L)

    # ---- per-core int16 gather indices ----
    pad_safe = np.where(positions >= 0, positions, 0)
    rown = row[pad_safe]
    coln = col[pad_safe]
    # hi-half buckets gather from a base offset by T_SPLIT
    boff = np.zeros(L, dtype=np.int64)
    o = 0
    for b in range(4):
        if b >= 2:
            boff[o:o + caps[b]] += T_SPLIT
        o += caps[b]
    rown = rown - boff[None, :]
    o = 0
    coff = np.zeros(L, dtype=np.int64)
    for b in range(4):
        if b % 2 == 1:
            coff[o:o + caps[b]] += T_SPLIT
        o += caps[b]
    coln = coln - coff[None, :]
    # padding slots point at node 0 of whatever base the bucket uses
    rown[positions < 0] = 0
    coln[positions < 0] = 0
    assert rown.min() >= 0 and rown.max() < T_SPLIT
    assert coln.min() >= 0 and coln.max() < T_SPLIT

    idx_row = [_wrap16(rown[c].astype(np.int16), L) for c in range(NCORES)]
    idx_col = [_wrap16(coln[c].astype(np.int16), L) for c in range(NCORES)]

    # ---- weights / biases ----
    bf16 = ml_dtypes.bfloat16
    xb = x.astype(bf16)
    w1a = W1[:N_FEAT].astype(bf16)
    w1b = W1[N_FEAT:].astype(bf16)
    w2v = W2.astype(bf16)
    w3v = W3.astype(bf16)
    b1v = np.ascontiguousarray(
        b1.astype(np.float32).reshape(2, N_FEAT).T)  # [:, j] = b1[j*128:...]
    b2v = b2.astype(np.float32).reshape(H2, 1)
    b3v = b3.astype(np.float32).reshape(1, 1)

    nc = _build(caps)

    in_maps = []
    for c in range(NCORES):
        in_maps.append({
            "xb": xb, "w1a": w1a, "w1b": w1b, "w2": w2v, "w3": w3v,
            "b1": b1v, "b2": b2v, "b3": b3v,
            "idxr": idx_row[c], "idxc": idx_col[c],
        })

    res = run_bass_kernel_spmd(nc, in_maps, list(range(NCORES)),
                               trace=PROFILE)
    LAST_EXEC_NS = res.exec_time_ns
    LAST_RESULTS = res

    out = np.zeros(N_EDGES, dtype=np.float32)
    for c in range(NCORES):
        vals = res.results[c]["y"][0]
        m = positions[c] >= 0
        out[positions[c][m]] = vals[m]
    return out


# revision 17
# speedup vs baseline: 1.2215x; 1.2215x over previous
"""Trainium2 Bass kernel for edge-MLP GNN message passing.

Computation (per edge e with endpoints row[e], col[e]):
    h1 = relu([x[row] | x[col]] @ W1 + b1)        # [E, 256]
    h2 = leaky_relu(h1 @ W2 + b2, 0.01)           # [E, 128]
    out = h2 @ W3 + b3                            # [E]

Strategy: data-parallel over edges across 8 NeuronCores. On each core the
row/col feature gathers use the SWDGE dma_gather custom instruction (bf16,
non-transpose) rotated across 4 SWDGE queues so all four Q7 cpu pairs
generate descriptors in parallel; gathered [edge, feat] blocks are
transposed to [feat, edge] on the TensorEngine. dma_gather indices are
int16, so edges
are bucketed on the host into 4 groups by (row >= 32768, col >= 32768); the
hi-half buckets gather from a base AP offset by 32768 rows.
"""

import numpy as np
import ml_dtypes

import bass_rust
import concourse.bass as bass
import concourse.mybir as mybir
import concourse.tile as tile
from concourse.bass_utils import run_bass_kernel_spmd
from concourse.library_config import mlp as mlp_library
from concourse.tile_rust import add_dep_helper
from concourse.vector_clock import ScopedClock

BF16 = mybir.dt.bfloat16
F32 = mybir.dt.float32
I16 = mybir.dt.int16

N_NODES = 50000
N_FEAT = 128
N_EDGES = 600000
HIDDEN = 256
H2 = 128
NCORES = 8
T_SPLIT = 32768  # int16 index limit +1
GCHUNK = 2048    # edges per dma_gather instruction
MCHUNK = 512     # edges per matmul group (PSUM free-dim limit for f32)
LEAKY = 0.01

PROFILE = False
LAST_EXEC_NS = None
LAST_RESULTS = None

_PATCHED = False


def _patch_tile_drain():
    """Upstream TileContext attaches every global-clock wait to the single
    final InstDrain, but non-EventSemaphore instructions encode at most one
    wait and walrus rejects the overfull drain. Spread the waits over
    dedicated sync wait instructions instead."""
    global _PATCHED
    if _PATCHED:
        return
    _PATCHED = True

    def _wait_cap(inst):
        # walrus encodes 2 sync waits on EventSemaphore, 1 elsewhere
        return 2 if "EventSemaphore" in type(inst).__name__ else 1

    def _split_overfull_waits(self, nc):
        sem_by_name = {}
        for k, h in self.sems.allocated().items():
            sem_by_name[getattr(h, "name", k)] = h
        cur = nc.cur_bb.bb
        for f in nc.m.functions:
            for bb in f.blocks:
                insts = bb.instructions
                i = 0
                while i < len(insts):
                    inst = insts[i]
                    si = inst.sync_info
                    waits = list(si.on_wait) if si is not None else []
                    cap = _wait_cap(inst)
                    if len(waits) <= cap:
                        i += 1
                        continue
                    keep, extra = waits[:cap], waits[cap:]
                    inst.sync_info = bass_rust.SyncInfo(
                        on_wait=keep, on_update=list(si.on_update)
                    )
                    carriers = []
                    for w in extra:
                        assert w.wait_reg is None, "register waits unsupported"
                        nc.engines[inst.engine].wait_ge(
                            sem_by_name[w.ant_name], w.wait_value
                        )
                        carriers.append(cur.instructions.pop())
                    for c in reversed(carriers):
                        insts.insert(i, c)
                    i += 1 + len(carriers)

    def _drain_and_barrier(self, tick_clock, wait_clock):
        nc = self.nc
        drain_inst = nc.sync.drain()
        wait_clock.add_sem_waits(
            drain_inst.ins, ScopedClock({None: tick_clock.global_clock})
        )
        nc.all_engine_barrier()
        _split_overfull_waits(self, nc)
        popped = nc._tile_sem_poison_stack.pop()
        assert popped is self._sem_poison
        nc.clear_and_free_semaphores(list(self.sems.allocated().values()))
        nc.all_engine_barrier()

    tile.TileContext._drain_and_barrier = _drain_and_barrier


def _gather_schedule(cap):
    """Split a bucket capacity (multiple of MCHUNK) into gather sizes."""
    out = [GCHUNK] * (cap // GCHUNK)
    rem = cap % GCHUNK
    if rem:
        out.append(rem)
    return out


_BUILD_CACHE = {}


def _build(caps):
    """Build the SPMD Bass module for per-core bucket capacities `caps`
    (tuple of 4 ints, each a multiple of MCHUNK)."""
    if caps in _BUILD_CACHE:
        return _BUILD_CACHE[caps]
    _patch_tile_drain()

    L = sum(caps)          # padded edges per core
    L16 = L // 16

    nc = bass.Bass("TRN2", target_bir_lowering=False, debug=False,
                   num_devices=NCORES, num_swdge_queues=4)

    xb = nc.dram_tensor("xb", [N_NODES, N_FEAT], BF16, kind="ExternalInput")
    w1a = nc.dram_tensor("w1a", [N_FEAT, HIDDEN], BF16, kind="ExternalInput")
    w1b = nc.dram_tensor("w1b", [N_FEAT, HIDDEN], BF16, kind="ExternalInput")
    w2 = nc.dram_tensor("w2", [HIDDEN, H2], BF16, kind="ExternalInput")
    w3 = nc.dram_tensor("w3", [H2, 1], BF16, kind="ExternalInput")
    b1 = nc.dram_tensor("b1", [N_FEAT, 2], F32, kind="ExternalInput")
    b2 = nc.dram_tensor("b2", [H2, 1], F32, kind="ExternalInput")
    b3 = nc.dram_tensor("b3", [1, 1], F32, kind="ExternalInput")
    idxr = nc.dram_tensor("idxr", [128, L16], I16, kind="ExternalInput")
    idxc = nc.dram_tensor("idxc", [128, L16], I16, kind="ExternalInput")
    y = nc.dram_tensor("y", [1, L], F32, kind="ExternalOutput")

    with tile.TileContext(nc) as tc:
        with (
            tc.tile_pool(name="const", bufs=1) as cpool,
            tc.tile_pool(name="gather", bufs=3) as gpool,
            tc.tile_pool(name="h", bufs=3) as hpool,
            tc.tile_pool(name="out", bufs=4) as opool,
            tc.tile_pool(name="psum", bufs=2, space="PSUM") as ppool,
            tc.tile_pool(name="psumtr", bufs=4, space="PSUM") as ppool_tr,
        ):
            # ---- preload constants ----
            w1a_sb = cpool.tile([N_FEAT, HIDDEN], BF16, tag="w1a")
            nc.sync.dma_start(out=w1a_sb[:], in_=w1a[:])
            w1b_sb = cpool.tile([N_FEAT, HIDDEN], BF16, tag="w1b")
            nc.sync.dma_start(out=w1b_sb[:], in_=w1b[:])
            w2_sb = cpool.tile([HIDDEN // 2, 2 * H2], BF16, tag="w2")
            # W2 is [256, 128] with contraction j on partitions; load as two
            # [128, 128] tiles side by side.
            nc.sync.dma_start(out=w2_sb[:, :H2], in_=w2[:128, :])
            nc.sync.dma_start(out=w2_sb[:, H2:], in_=w2[128:, :])
            w3_sb = cpool.tile([H2, 1], BF16, tag="w3")
            nc.sync.dma_start(out=w3_sb[:], in_=w3[:])
            b1_sb = cpool.tile([N_FEAT, 2], F32, tag="b1")
            nc.sync.dma_start(out=b1_sb[:], in_=b1[:])
            b2_sb = cpool.tile([H2, 1], F32, tag="b2")
            nc.sync.dma_start(out=b2_sb[:], in_=b2[:])
            b3_sb = cpool.tile([1, 1], F32, tag="b3")
            nc.sync.dma_start(out=b3_sb[:], in_=b3[:])
            ident = cpool.tile([128, 128], BF16, tag="ident")
            from concourse.masks import make_identity
            make_identity(nc, ident[:])
            idxr_sb = cpool.tile([128, L16], I16, tag="idxr")
            nc.sync.dma_start(out=idxr_sb[:], in_=idxr[:])
            idxc_sb = cpool.tile([128, L16], I16, tag="idxc")
            nc.sync.dma_start(out=idxc_sb[:], in_=idxc[:])

            # dma_gather runs as Q7 ucode from the mlp library; the reload
            # must execute on Pool before any gather (Tile won't order it
            # by data deps, so wire explicit edges)
            lib_inst = nc.gpsimd.load_library(mlp_library).ins

            # one register per distinct gather size (dma_gather's
            # num_idxs_reg); allocating per-call exhausts Pool registers
            reg_cache = {}
            qctr = [0]

            def gsize_reg(v):
                if v not in reg_cache:
                    reg_cache[v] = nc.gpsimd.to_reg(v)
                return reg_cache[v]

            off = 0
            for b in range(4):
                row_base = xb[:] if b < 2 else xb[T_SPLIT:, :]
                col_base = xb[:] if b % 2 == 0 else xb[T_SPLIT:, :]
                for gsize in _gather_schedule(caps[b]):
                    c0 = off // 16
                    c1 = (off + gsize) // 16
                    rowG = gpool.tile([128, GCHUNK // 128, N_FEAT], BF16,
                                      tag="rowG")
                    g1 = nc.gpsimd.dma_gather(
                        rowG[:, :gsize // 128, :],
                        row_base,
                        idxr_sb[:, c0:c1],
                        num_idxs=gsize,
                        num_idxs_reg=gsize_reg(gsize),
                        elem_size=N_FEAT,
                        transpose=False,
                        single_packet=False,
                        queue_num=qctr[0] % 4,
                    )
                    qctr[0] += 1
                    colG = gpool.tile([128, GCHUNK // 128, N_FEAT], BF16,
                                      tag="colG")
                    g2 = nc.gpsimd.dma_gather(
                        colG[:, :gsize // 128, :],
                        col_base,
                        idxc_sb[:, c0:c1],
                        num_idxs=gsize,
                        num_idxs_reg=gsize_reg(gsize),
                        elem_size=N_FEAT,
                        transpose=False,
                        single_packet=False,
                        queue_num=qctr[0] % 4,
                    )
                    qctr[0] += 1
                    add_dep_helper(g1.ins, lib_inst, sync=False,
                                   reason="gather needs mlp lib")
                    add_dep_helper(g2.ins, lib_inst, sync=False,
                                   reason="gather needs mlp lib")
                    for s in range(gsize // MCHUNK):
                        rowT = hpool.tile([128, MCHUNK], BF16, tag="rowT")
                        colT = hpool.tile([128, MCHUNK], BF16, tag="colT")
                        for i in range(MCHUNK // 128):
                            blk = s * (MCHUNK // 128) + i
                            ptr = ppool_tr.tile([128, 128], BF16, tag="ptr")
                            nc.tensor.transpose(ptr[:], rowG[:, blk, :],
                                                ident[:])
                            dst = rowT[:, i * 128:(i + 1) * 128]
                            if i % 2 == 0:
                                nc.scalar.copy(dst, ptr[:])
                            else:
                                nc.vector.tensor_copy(dst, ptr[:])
                            ptc = ppool_tr.tile([128, 128], BF16, tag="ptr")
                            nc.tensor.transpose(ptc[:], colG[:, blk, :],
                                                ident[:])
                            dst = colT[:, i * 128:(i + 1) * 128]
                            if i % 2 == 0:
                                nc.vector.tensor_copy(dst, ptc[:])
                            else:
                                nc.scalar.copy(dst, ptc[:])
                        rT = rowT[:]
                        cT = colT[:]
                        # ---- layer 1: h1[j, e] ----
                        ps_h1_0 = ppool.tile([128, MCHUNK], F32, tag="ph1a")
                        nc.tensor.matmul(ps_h1_0[:], w1a_sb[:, :128], rT,
                                         start=True, stop=False)
                        nc.tensor.matmul(ps_h1_0[:], w1b_sb[:, :128], cT,
                                         start=False, stop=True)
                        ps_h1_1 = ppool.tile([128, MCHUNK], F32, tag="ph1b")
                        nc.tensor.matmul(ps_h1_1[:], w1a_sb[:, 128:], rT,
                                         start=True, stop=False)
                        nc.tensor.matmul(ps_h1_1[:], w1b_sb[:, 128:], cT,
                                         start=False, stop=True)
                        h1_0 = hpool.tile([128, MCHUNK], BF16, tag="h1a")
                        nc.scalar.activation(
                            h1_0[:], ps_h1_0[:],
                            mybir.ActivationFunctionType.Relu,
                            bias=b1_sb[:, 0:1])
                        h1_1 = hpool.tile([128, MCHUNK], BF16, tag="h1b")
                        nc.vector.tensor_scalar(
                            h1_1[:], ps_h1_1[:], b1_sb[:, 1:2], 0.0,
                            op0=mybir.AluOpType.add,
                            op1=mybir.AluOpType.max)
                        # ---- layer 2: h2[m, e] ----
                        ps_h2 = ppool.tile([128, MCHUNK], F32, tag="ph1a")
                        nc.tensor.matmul(ps_h2[:], w2_sb[:, :H2], h1_0[:],
                                         start=True, stop=False)
                        nc.tensor.matmul(ps_h2[:], w2_sb[:, H2:], h1_1[:],
                                         start=False, stop=True)
                        z = hpool.tile([128, MCHUNK], BF16, tag="z")
                        nc.vector.tensor_scalar(
                            z[:], ps_h2[:], b2_sb[:, 0:1], None,
                            op0=mybir.AluOpType.add)
                        h2 = hpool.tile([128, MCHUNK], BF16, tag="h2")
                        nc.vector.scalar_tensor_tensor(
                            h2[:], z[:], LEAKY, z[:],
                            op0=mybir.AluOpType.mult,
                            op1=mybir.AluOpType.max)
                        # ---- layer 3: out[e] ----
                        ps_out = ppool.tile([1, MCHUNK], F32, tag="ph1b")
                        nc.tensor.matmul(ps_out[:], w3_sb[:], h2[:],
                                         start=True, stop=True)
                        o_sb = opool.tile([1, MCHUNK], F32, tag="osb")
                        nc.scalar.activation(
                            o_sb[:], ps_out[:],
                            mybir.ActivationFunctionType.Identity,
                            bias=b3_sb[:, 0:1])
                        nc.sync.dma_start(
                            out=y[:, off + s * MCHUNK: off + (s + 1) * MCHUNK],
                            in_=o_sb[:])
                    off += gsize

    # populate .instr bytes for extended-inst InstISA subclasses (the
    # library-reload op); raw Bass skips this Bacc pass
    from concourse.library_overlay import lower_extended_insts
    lower_extended_insts(nc)

    _BUILD_CACHE[caps] = nc
    return nc


def _wrap16(arr, L):
    """[L] -> [128, L//16] gather-index layout: element i at [i%16, i//16],
    replicated across the 8 GPSIMD core partition groups."""
    w = arr.reshape(L // 16, 16).T  # [16, L//16]
    return np.ascontiguousarray(np.tile(w, (8, 1)))


def kernel(x, edge_index, W1, b1, W2, b2, W3, b3):
    global LAST_EXEC_NS, LAST_RESULTS
    x = np.asarray(x)
    edge_index = np.asarray(edge_index)
    row = edge_index[0].astype(np.int64)
    col = edge_index[1].astype(np.int64)

    # ---- bucket edges by (row_hi, col_hi) ----
    key = (row >= T_SPLIT) * 2 + (col >= T_SPLIT)
    order = np.argsort(key, kind="stable")
    sizes = np.bincount(key, minlength=4)

    per_core_bucket = []
    caps = []
    start = 0
    for b in range(4):
        ids = order[start:start + sizes[b]]
        start += sizes[b]
        share = -(-sizes[b] // NCORES)  # ceil
        cap = -(-share // MCHUNK) * MCHUNK
        caps.append(int(cap))
        padded = np.full(NCORES * cap, -1, dtype=np.int64)
        # distribute contiguous slices of size `share`, then pad each to cap
        for c in range(NCORES):
            sl = ids[c * share:(c + 1) * share]
            padded[c * cap:c * cap + len(sl)] = sl
        per_core_bucket.append(padded.reshape(NCORES, cap))
    caps = tuple(caps)
    L = sum(caps)

    positions = np.concatenate(per_core_bucket, axis=1)  # [NCORES, L]
    assert positions.shape == (NCORES, L)

    # ---- per-core int16 gather indices ----
    pad_safe = np.where(positions >= 0, positions, 0)
    rown = row[pad_safe]
    coln = col[pad_safe]
    # hi-half buckets gather from a base offset by T_SPLIT
    boff = np.zeros(L, dtype=np.int64)
    o = 0
    for b in range(4):
        if b >= 2:
            boff[o:o + caps[b]] += T_SPLIT
        o += caps[b]
    rown = rown - boff[None, :]
    o = 0
    coff = np.zeros(L, dtype=np.int64)
    for b in range(4):
        if b % 2 == 1:
            coff[o:o + caps[b]] += T_SPLIT
        o += caps[b]
    coln = coln - coff[None, :]
    # padding slots point at node 0 of whatever base the bucket uses
    rown[positions < 0] = 0
    coln[positions < 0] = 0
    assert rown.min() >= 0 and rown.max() < T_SPLIT
    assert coln.min() >= 0 and coln.max() < T_SPLIT

    idx_row = [_wrap16(rown[c].astype(np.int16), L) for c in range(NCORES)]
    idx_col = [_wrap16(coln[c].astype(np.int16), L) for c in range(NCORES)]

    # ---- weights / biases ----
    bf16 = ml_dtypes.bfloat16
    xb = x.astype(bf16)
    w1a = W1[:N_FEAT].astype(bf16)
    w1b = W1[N_FEAT:].astype(bf16)
    w2v = W2.astype(bf16)
    w3v = W3.astype(bf16)
    b1v = np.ascontiguousarray(
        b1.astype(np.float32).reshape(2, N_FEAT).T)  # [:, j] = b1[j*128:...]
    b2v = b2.astype(np.float32).reshape(H2, 1)
    b3v = b3.astype(np.float32).reshape(1, 1)

    nc = _build(caps)

    in_maps = []
    for c in range(NCORES):
        in_maps.append({
            "xb": xb, "w1a": w1a, "w1b": w1b, "w2": w2v, "w3": w3v,
            "b1": b1v, "b2": b2v, "b3": b3v,
            "idxr": idx_row[c], "idxc": idx_col[c],
        })

    res = run_bass_kernel_spmd(nc, in_maps, list(range(NCORES)),
                               trace=PROFILE)
    LAST_EXEC_NS = res.exec_time_ns
    LAST_RESULTS = res

    out = np.zeros(N_EDGES, dtype=np.float32)
    for c in range(NCORES):
        vals = res.results[c]["y"][0]
        m = positions[c] >= 0
        out[positions[c][m]] = vals[m]
    return out
